# revision 50
# baseline (speedup 1.0000x reference)
"""Trainium2 Bass kernel for one transformer decoder block
(LN -> causal self-attn -> LN -> cross-attn -> LN -> MLP, residuals),
data-parallel over batch: 8 batch elements -> 8 NeuronCores, no collectives.

v3: fp8 (e4m3) DoubleRow matmuls for every weight projection (K=256 per
matmul, 0.5 cycles/row), with activations quantized on-chip at scale 16
and weights host-quantized at scale 256 (PSUM readouts fold the 1/4096).
The MLP — the only error-sensitive projections — runs compensated
DoubleRow groups: fc uses W_hi*A_hi + W_lo*A_hi + W_hi*A_lo (near-bf16
accuracy), mo uses W_hi*A_hi + W_lo*A_hi; all other projections run
plain fp8.  Measured block rel err vs the fp32 reference: 1.24e-2.  Attention: scores bf16, causal mask
applied as a -2400 psum-add matmul (exp underflows to 0), P = exp scores
written fp8 by ACT, V fp8, P@V DoubleRow over s-tile pairs with the
all-ones 65th V column producing the softmax denominator; denominator
reciprocals broadcast with one K=2 block-expander matmul per head pair.

The two 512-token chunk streams are software-pipelined: attention
emission (ACT-bound on exp) interleaves "filler" PE projection units of
the other chunk between head pairs, so the in-order PE stream has work
while ACT drains exps.  LayerNorms of one chunk are emitted as filler
inside the other chunk's attention.
"""

import contextlib

import numpy as np
import ml_dtypes

import concourse.bass as bass
import concourse.tile as tile
from concourse import mybir
from concourse.bass_utils import run_bass_kernel_spmd
from concourse.vector_clock import ScopedClock, VectorClock

F32 = mybir.dt.float32
BF16 = mybir.dt.bfloat16
F8 = mybir.dt.float8e4
F8E5 = mybir.dt.float8e5
AF = mybir.ActivationFunctionType
DR = mybir.MatmulPerfMode.DoubleRow
ALU = mybir.AluOpType

B, T, C, H = 8, 1024, 1024, 16
I, IP = 257, 384            # encoder tokens, padded to 3 s-tiles
KT = C // 128               # 8 k-tiles over the 1024 contraction
FC = 4 * C
KT_FC = FC // 128           # 32
CHW = 512                   # token chunk width
NCH = T // CHW              # 2
ST = T // 128               # self-attn s-tiles
SI = IP // 128              # cross-attn s-tiles (3)
SW = 256.0                  # weight fp8 scale
SA = 16.0                   # activation fp8 scale
PS_INV = 1.0 / (SW * SA)    # psum rescale for (16x act) @ (256x weight)

# --------------------------------------------------------------------------
# Workaround: this walrus build rejects >1 sync wait per instruction, but
# Tile's wait-assignment can attach several.  Split extras onto nofuse NoOps
# placed just before the instruction on the same engine, and emit the exit
# drain's per-proc waits as individual single-wait nops.
# --------------------------------------------------------------------------
_MAX_WAITS = 1
_orig_lower = tile.TileContext._lower_ordered_insts


def _split_waits(insts):
    out = []
    for inst in insts:
        si = getattr(inst, "sync_info", None)
        waits = list(si.on_wait) if si is not None and si.on_wait else []
        if len(waits) > _MAX_WAITS:
            spill, keep = waits[:-_MAX_WAITS], waits[-_MAX_WAITS:]
            for j, w in enumerate(spill):
                out.append(mybir.InstNoOp(
                    name=f"{inst.name}_ws{j}",
                    sync_info=mybir.SyncInfo(on_wait=[w], on_update=[]),
                    bass_nofuse=True,
                    engine=inst.engine,
                ))
            inst.sync_info = mybir.SyncInfo(on_wait=keep,
                                            on_update=list(si.on_update))
        out.append(inst)
    return out


def _patched_lower(self, ordered):
    for bb_name, insts in list(ordered.items()):
        ordered[bb_name] = _split_waits(insts)
    return _orig_lower(self, ordered)


def _patched_drain_and_barrier(self, tick_clock, wait_clock):
    gc = tick_clock.global_clock
    for p in range(len(gc)):
        t = gc[p]
        if t <= 0:
            continue
        vc = VectorClock()
        vc.require_at_least(p, t)
        w = self.nc.sync.nop(nofuse=True, hint=f"drain_split_p{p}")
        wait_clock.add_sem_waits(w.ins, ScopedClock({None: vc}))
    self.nc.sync.drain()
    self.nc.all_engine_barrier()
    assert self.sems is not None
    popped = self.nc._tile_sem_poison_stack.pop()
    assert popped is self._sem_poison
    self.nc.clear_and_free_semaphores(list(self.sems.allocated().values()))
    self.nc.all_engine_barrier()


tile.TileContext._lower_ordered_insts = _patched_lower
tile.TileContext._drain_and_barrier = _patched_drain_and_barrier


# --------------------------------------------------------------------------
# Kernel builder (single NeuronCore program, run SPMD on 8 cores)
# --------------------------------------------------------------------------
CH_A = (0, 0, CHW)
CH_B = (1, CHW, CHW)
TCH = [CH_A, CH_B]
ECH = [(0, 0, IP)]


def _flat(t3):
    """[p, a, b] tile -> [p, a*b] AP over the same storage."""
    p, a, b = t3.shape
    return bass.AP(tensor=t3.tensor, offset=t3.offset,
                   ap=[list(t3.ap[0]), [1, a * b]])


def _emit(nc, tc, dd, o, flags):
    (bq_nz, bq2_nz, bkv_nz, bao_nz, bco_nz, bfc_nz, bmo_nz, ln_trivial) = flags
    ctx = contextlib.ExitStack()
    with ctx:
        consts = ctx.enter_context(tc.tile_pool(name="consts", bufs=1))
        resid = ctx.enter_context(tc.tile_pool(name="resid", bufs=1))
        acts = ctx.enter_context(tc.tile_pool(name="acts", bufs=1))
        wpool = ctx.enter_context(tc.tile_pool(name="wpool", bufs=2))
        wpfc = ctx.enter_context(tc.tile_pool(name="wpfc", bufs=2))
        wpmo = ctx.enter_context(tc.tile_pool(name="wpmo", bufs=2))
        tmps = ctx.enter_context(tc.tile_pool(name="tmps", bufs=2))
        scp = ctx.enter_context(tc.tile_pool(name="scp", bufs=7))
        nrm = ctx.enter_context(tc.tile_pool(name="nrm", bufs=2))
        psA = ctx.enter_context(tc.tile_pool(name="psA", bufs=2, space="PSUM"))
        psB = ctx.enter_context(tc.tile_pool(name="psB", bufs=2, space="PSUM"))
        psC = ctx.enter_context(tc.tile_pool(name="psC", bufs=2, space="PSUM"))

        # ---------------- constants ----------------
        def colvec(name, src_ap, n):
            t = consts.tile([128, n], F32, tag=name)
            nc.sync.dma_start(out=t, in_=src_ap.rearrange("(n p) -> p n", p=128))
            return t

        bias_qk = colvec("bias_qk", dd["b_qkv"][0:2 * C], 16)
        bias_q = colvec("bias_q", dd["b_q"][:], 8)
        bias_kvk = colvec("bias_kvk", dd["b_kv"][0:C], 8)
        bias_fc = colvec("bias_fc", dd["b_fc"][:], 32)
        g1 = colvec("g1", dd["ln1_g"][:], 8)
        b1 = colvec("b1", dd["ln1_b"][:], 8)
        g2 = colvec("g2", dd["ln2_g"][:], 8)
        b2 = colvec("b2", dd["ln2_b"][:], 8)
        g3 = colvec("g3", dd["ln3_g"][:], 8)
        b3 = colvec("b3", dd["ln3_b"][:], 8)
        bias_ao = colvec("bias_ao", dd["b_ao"][:], 8)
        bias_co = colvec("bias_co", dd["b_co"][:], 8)
        bias_mo = colvec("bias_mo", dd["b_mo"][:], 8)

        # free-axis bias tiles (broadcast across partitions) for V biases,
        # pre-scaled by SA on the host; only loaded when nonzero.
        def bcast_load(tag, src_ap):
            t = consts.tile([128, NCH, CHW], BF16, tag=tag)
            src = src_ap.rearrange("(c n) -> c n", c=NCH)
            nc.gpsimd.dma_start(out=t, in_=bass.AP(
                tensor=src.tensor, offset=src.offset,
                ap=[[0, 128]] + [list(a) for a in src.ap]))
            return t

        bvb = (bcast_load("bvb", dd["bvb16"][:]) if bq_nz else None)
        bvcb = (bcast_load("bvcb", dd["bvcb16"][:]) if bkv_nz else None)

        trineg = consts.tile([128, 128], BF16, tag="trineg")
        nc.sync.dma_start(out=trineg, in_=dd["trineg"][:, :])
        identm = consts.tile([128, 128], BF16, tag="identm")
        nc.sync.dma_start(out=identm, in_=dd["identm"][:, :])
        allneg = consts.tile([128, 128], BF16, tag="allneg")
        nc.sync.dma_start(out=allneg, in_=dd["allneg"][:, :])
        smask = consts.tile([128, 1], F32, tag="smask")
        nc.sync.dma_start(out=smask, in_=dd["smask"][:, :])
        onesc = consts.tile([128, 128], BF16, tag="onesc")
        nc.sync.dma_start(out=onesc, in_=dd["onesc"][:, :])
        ebk = consts.tile([2, 128], BF16, tag="ebk")
        nc.sync.dma_start(out=ebk, in_=dd["ebk"][:, :])
        epsr = consts.tile([128, 1], F32, tag="epsr")
        nc.vector.memset(epsr, 1e-5 / (SA * SA))

        o_dst = o[:, :].rearrange("(kt p) t -> p kt t", p=128)
        # ---------------- residual stream + encoder ----------------
        xTb = acts.tile([128, KT, T], BF16, tag="big32", name="xTb")
        xTb_src = dd["xTb"][:, :].rearrange("(kt p) t -> p kt t", p=128)
        for k in range(KT):
            nc.sync.dma_start(out=xTb[:, k, :], in_=xTb_src[:, k, :])
        xT = resid.tile([128, KT, T], F32, tag="xT")
        xT_src = dd["xT"][:, :].rearrange("(kt p) t -> p kt t", p=128)
        encT = acts.tile([128, KT, IP], F8, tag="encT")

        # ---------------- pipelining queue ----------------
        pending = []

        def fill(n):
            for _ in range(n):
                if not pending:
                    return
                pending.pop(0)()

        def drain():
            while pending:
                pending.pop(0)()

        # ---------------- layernorm (one token chunk) ----------------
        def ln_chunk(src, g, b, chunk, hi, lo, src16=None):
            ci, c0, cw = chunk
            ps2s = psB.tile([128, 2 * CHW], F32, tag="psB", name="lnps")
            psu, psq = ps2s[:, 0:CHW], ps2s[:, CHW:2 * CHW]
            for k in range(KT):
                if src16 is None:
                    xb = acts.tile([128, CHW], BF16, tag="lnB", bufs=4,
                                   name="xb")
                    nc.gpsimd.tensor_copy(out=xb, in_=src[:, k, c0:c0 + cw])
                else:
                    xb = src16[:, k, c0:c0 + cw]
                sq = tmps.tile([128, CHW], BF16, tag="sq", bufs=2, name="sq")
                nc.vector.tensor_mul(out=sq, in0=xb, in1=xb)
                nc.tensor.matmul(psu, onesc, xb,
                                 start=(k == 0), stop=(k == KT - 1))
                nc.tensor.matmul(psq, onesc, sq,
                                 start=(k == 0), stop=(k == KT - 1))
            ab = tmps.tile([128, CHW], BF16, tag="ab", bufs=1, name="ab")
            mb = tmps.tile([128, CHW], BF16, tag="mb", bufs=1, name="mb")
            ex2 = tmps.tile([128, CHW], BF16, tag="ex2", bufs=1, name="ex2")
            nc.vector.tensor_scalar_mul(out=mb, in0=psu, scalar1=1.0 / C)
            nc.vector.tensor_scalar_mul(out=ex2, in0=psq, scalar1=1.0 / C)
            nc.vector.tensor_mul(out=ab, in0=mb, in1=mb)          # mu^2
            nc.vector.tensor_sub(out=ab, in0=ex2, in1=ab)         # var
            # sqrt((var+eps)/SA^2) -> rstd/SA; recip -> SA*rstd
            nc.scalar.activation(out=ab, in_=ab, func=AF.Sqrt,
                                 bias=epsr, scale=1.0 / (SA * SA))
            with nc.allow_low_precision(reason="bf16 rstd is plenty"):
                nc.vector.reciprocal(out=ab, in_=ab)
            nc.vector.tensor_mul(out=mb, in0=mb, in1=ab)
            for k in range(KT):
                t1 = tmps.tile([128, CHW], BF16, tag="lnt", bufs=2, name="t1")
                nc.vector.tensor_mul(
                    out=t1,
                    in1=ab,
                    in0=(src16[:, k, c0:c0 + cw] if src16 is not None
                         else src[:, k, c0:c0 + cw]))
                if ln_trivial:
                    if lo is not None:
                        s1 = tmps.tile([128, CHW], BF16, tag="s1", bufs=1,
                                       name="s1")
                        nc.vector.tensor_sub(out=s1, in0=t1, in1=mb)
                        nc.gpsimd.tensor_copy(out=hi[:, k, c0:c0 + cw], in_=s1)
                        nc.vector.tensor_sub(out=lo[:, k, c0:c0 + cw],
                                             in0=s1, in1=hi[:, k, c0:c0 + cw])
                    else:
                        nc.vector.tensor_sub(out=hi[:, k, c0:c0 + cw],
                                             in0=t1, in1=mb)
                else:
                    nc.vector.tensor_sub(out=t1, in0=t1, in1=mb)
                    s1 = tmps.tile([128, CHW], BF16, tag="s1", bufs=1, name="s1")
                    # ln_b arrives pre-scaled by SA from the host
                    nc.scalar.activation(out=s1, in_=t1, func=AF.Identity,
                                         bias=b[:, k:k + 1], scale=g[:, k:k + 1])
                    nc.gpsimd.tensor_copy(out=hi[:, k, c0:c0 + cw], in_=s1)
                    if lo is not None:
                        nc.vector.tensor_sub(out=lo[:, k, c0:c0 + cw],
                                             in0=s1, in1=hi[:, k, c0:c0 + cw])

        # ---------------- fp8 DoubleRow projection (unit emission) --------
        def proj_units(wgroups, col0, ncols, nk, chunks, cb, fbw=512, wp=None,
                       wtag="wb", pretiled=False):
            """Return a list of closures; each emits one psum block (or one
            weight-block DMA).  psum[f,t] = sum_g sum_k Wg[k,col0+f]*Ag[k,t]."""
            if wp is None:
                wp = wpool
            seen = {}
            for (w_ap, rhs3) in wgroups:
                key = id(w_ap.tensor)
                if key not in seen:
                    if pretiled:
                        seen[key] = w_ap
                    else:
                        seen[key] = w_ap.rearrange("(kt p) n -> p kt n", p=128)
            nkp = nk // 2
            total = len(wgroups) * nkp
            units = []
            for fb in range(ncols // fbw):
                wbs = {}

                def load(fb=fb, wbs=wbs):
                    for i, (key, wre) in enumerate(seen.items()):
                        wb = wp.tile([128, nk, fbw], F8, tag=f"{wtag}{i}",
                                     name=f"{wtag}{i}")
                        if pretiled:
                            nc.sync.dma_start(
                                out=wb,
                                in_=wre[fb * 128:(fb + 1) * 128, :].rearrange(
                                    "p (kt n) -> p kt n", n=fbw))
                        else:
                            nc.sync.dma_start(
                                out=wb, in_=wre[:, :, col0 + fb * fbw:
                                                col0 + (fb + 1) * fbw])
                        wbs[key] = wb
                units.append(load)
                for ci, c0, cw in chunks:
                    for fi in range(fbw // 128):
                        ftile = (fb * fbw) // 128 + fi
                        def unit(fi=fi, ftile=ftile, ci=ci, c0=c0, cw=cw,
                                 wbs=wbs):
                            ps = psA.tile([128, CHW], F32, tag="psA",
                                          name="ps")
                            idx = 0
                            for (w_ap, rhs3) in wgroups:
                                wb = wbs[id(w_ap.tensor)]
                                for kp in range(nkp):
                                    nc.tensor.matmul(
                                        ps[:, :cw],
                                        wb[:, 2 * kp:2 * kp + 2,
                                           fi * 128:(fi + 1) * 128],
                                        rhs3[:, 2 * kp:2 * kp + 2, c0:c0 + cw],
                                        start=(idx == 0),
                                        stop=(idx == total - 1),
                                        perf_mode=DR)
                                    idx += 1
                            cb(ps, ftile, ci, c0, cw)
                        units.append(unit)
            return units

        def emit(units):
            for u in units:
                u()

        # ---------------- V projection (unit emission) ----------------
        def vproj_load(w_ap, col0, holder):
            wre = w_ap.rearrange("(kt p) n -> p kt n", p=128)

            def load():
                for fb in range(2):
                    wb = wpool.tile([128, KT, CHW], F8, tag="wv", name="wv")
                    nc.sync.dma_start(
                        out=wb,
                        in_=wre[:, :, col0 + fb * CHW: col0 + (fb + 1) * CHW])
                    holder[fb] = wb
            return load

        def vproj_units(lhs3, s_range, dst, holder, bias_t=None,
                        pad_mask=None, last_s=None):
            units = []
            for s in s_range:
                def unit(s=s):
                    dv = dst[:, s, :].rearrange("p (h e) -> p h e", e=65)
                    for fb in range(2):
                        ps = psA.tile([128, CHW], F32, tag="psA", name="vps")
                        for kp in range(KT // 2):
                            nc.tensor.matmul(
                                ps,
                                lhs3[:, 2 * kp:2 * kp + 2,
                                     s * 128:(s + 1) * 128],
                                holder[fb][:, 2 * kp:2 * kp + 2, :],
                                start=(kp == 0), stop=(kp == KT // 2 - 1),
                                perf_mode=DR)
                        if bias_t is None:
                            nc.scalar.activation(
                                out=dv[:, 8 * fb:8 * fb + 8, 0:64],
                                in_=ps.rearrange("p (h d) -> p h d", d=64),
                                func=AF.Copy, scale=SA * PS_INV)
                        else:
                            vtmp = tmps.tile([128, CHW], BF16, tag="vtmp",
                                             name="vtmp")
                            nc.scalar.activation(out=vtmp, in_=ps, func=AF.Copy,
                                                 scale=SA * PS_INV)
                            nc.vector.tensor_add(
                                out=dv[:, 8 * fb:8 * fb + 8, 0:64],
                                in0=vtmp.rearrange("p (h d) -> p h d", d=64),
                                in1=bias_t[:, fb, :].rearrange(
                                    "p (h d) -> p h d", d=64))
                    nc.gpsimd.memset(dv[:, :, 64:65], 1.0)
                    if pad_mask is not None and s == last_s:
                        nc.vector.tensor_scalar_mul(out=dst[:, s, :],
                                                    in0=dst[:, s, :],
                                                    scalar1=pad_mask)
                units.append(unit)
            return units

        # ---------------- attention (one token chunk) ----------------
        def attention(q3, k3, vsb_, chunk, causal, dst, per_pair=0):
            ci, c0, cw = chunk

            def scores_one(h):
                po, ft = (h % 2) * 64, h // 2
                s_list = list(range(4 * (ci + 1))) if causal else list(range(SI))
                pairs = []
                for i0 in range(0, len(s_list) - 1, 2):
                    pairs.append((s_list[i0], s_list[i0 + 1]))
                single = s_list[-1] if len(s_list) % 2 else None
                sc_info = []
                for (sga, sgb) in pairs:
                    offa = max(sga * 128 - c0, 0) if causal else 0
                    offb = max(sgb * 128 - c0, 0) if causal else 0
                    ps2 = psB.tile([128, 2 * CHW], F32, tag="psB", name="ps2")
                    sc2 = scp.tile([128, 2, CHW], F8, tag="sc", name="sc2")
                    for j, (sg, off) in enumerate(((sga, offa), (sgb, offb))):
                        diag = causal and sg * 128 - c0 >= 0
                        nc.tensor.matmul(
                            ps2[:, j * CHW + off:j * CHW + cw],
                            k3[po:po + 64, ft, sg * 128:(sg + 1) * 128],
                            q3[po:po + 64, ft, c0 + off:c0 + cw],
                            start=True, stop=not diag)
                        if diag:
                            # -2400 upper-triangle add; exp(0.125*...) -> 0
                            nc.tensor.matmul(
                                ps2[:, j * CHW + off:j * CHW + off + 128],
                                trineg, identm, start=False, stop=True)
                    if offb > offa:
                        nc.tensor.matmul(ps2[:, CHW + offa:CHW + offb],
                                         allneg, identm[:, 0:offb - offa],
                                         start=True, stop=True)
                    scf = _flat(sc2)
                    nc.scalar.activation(out=scf[:, offa:CHW + cw],
                                         in_=ps2[:, offa:CHW + cw],
                                         func=AF.Exp, scale=0.125)
                    sc_info.append(("pair", (sga, sgb), offa, sc2))
                if single is not None:
                    off = max(single * 128 - c0, 0) if causal else 0
                    ps2 = psB.tile([128, 2 * CHW], F32, tag="psB", name="ps1")
                    sc2 = scp.tile([128, 2, CHW], F8, tag="sc", name="sc1")
                    diag = causal and single * 128 - c0 >= 0
                    nc.tensor.matmul(
                        ps2[:, off:cw],
                        k3[po:po + 64, ft, single * 128:(single + 1) * 128],
                        q3[po:po + 64, ft, c0 + off:c0 + cw],
                        start=True, stop=not diag)
                    if diag:
                        nc.tensor.matmul(ps2[:, off:off + 128],
                                         trineg, identm, start=False, stop=True)
                    nc.scalar.activation(out=sc2[:, 0, off:cw],
                                         in_=ps2[:, off:cw],
                                         func=AF.Exp, scale=0.125)
                    sc_info.append(("single", (single,), off, sc2))
                return sc_info

            def pv_one(h, sc_info, rinv2, j):
                pv = psC.tile([128, CHW], F32, tag="psC", name="pv")
                n = len(sc_info)
                for idx, (kind, sgs, off, sc2) in enumerate(sc_info):
                    st, sp = (idx == 0), (idx == n - 1)
                    if kind == "pair":
                        nc.tensor.matmul(
                            pv[0:65, off:cw],
                            vsb_[:, sgs[0]:sgs[0] + 2, h * 65:(h + 1) * 65],
                            sc2[:, :, off:cw],
                            start=st, stop=sp, perf_mode=DR)
                    else:
                        nc.tensor.matmul(
                            pv[0:65, off:cw],
                            vsb_[:, sgs[0], h * 65:(h + 1) * 65],
                            sc2[:, 0, off:cw],
                            start=st, stop=sp)
                with nc.allow_low_precision(reason="bf16 softmax denom"):
                    nc.vector.reciprocal(out=rinv2[:, :cw],
                                         in_=pv[64:65, :cw])
                return pv

            for hp in range(0, H, 2):
                infos = []
                for h in (hp, hp + 1):
                    infos.append((h, scores_one(h)))
                rinvs = [nrm.tile([1, CHW], BF16, tag="rinv2", name="rinva"),
                         nrm.tile([1, CHW], BF16, tag="rinv3", name="rinvb")]
                pvs = []
                for j, (h, sc_info) in enumerate(infos):
                    pvs.append(pv_one(h, sc_info, rinvs[j], j))
                rps = psA.tile([128, CHW], F32, tag="psA", name="rps")
                for j in range(2):
                    nc.tensor.matmul(rps[64 * j:64 * (j + 1), :cw],
                                     onesc[0:1, 0:64], rinvs[j][:, :cw],
                                     start=True, stop=True)
                rbs = nrm.tile([128, CHW], BF16, tag="rb", name="rbs")
                nc.vector.tensor_copy(out=rbs[:, :cw], in_=rps[:, :cw])
                for j, (h, _si) in enumerate(infos):
                    po, ft = (h % 2) * 64, h // 2
                    nc.vector.tensor_mul(
                        out=dst[po:po + 64, ft, c0:c0 + cw],
                        in0=pvs[j][0:64, :cw],
                        in1=rbs[64 * j:64 * (j + 1), :cw])
                fill(per_pair)

        # ---------------- residual add callbacks ----------------
        def resid_cb(bias_t, use_bias, scale, store_out=False):
            def cb(ps, ftile, ci, c0, cw):
                if use_bias:
                    rtmp = tmps.tile([128, CHW], BF16, tag="rtmp", name="rtmp")
                    nc.scalar.activation(out=rtmp[:, :cw], in_=ps[:, :cw],
                                         func=AF.Identity, scale=scale,
                                         bias=bias_t[:, ftile:ftile + 1])
                    nc.vector.tensor_add(out=xT[:, ftile, c0:c0 + cw],
                                         in0=xT[:, ftile, c0:c0 + cw],
                                         in1=rtmp[:, :cw])
                else:
                    nc.vector.scalar_tensor_tensor(
                        out=xT[:, ftile, c0:c0 + cw], in0=ps[:, :cw],
                        scalar=scale, in1=xT[:, ftile, c0:c0 + cw],
                        op0=ALU.mult, op1=ALU.add)
                if store_out and ci == NCH - 1:
                    nc.sync.dma_start(out=o_dst[:, ftile, :],
                                      in_=xT[:, ftile, :])
            return cb

        # ================= block body =================
        # ---- sublayer 1: causal self-attention (chunk-pipelined) ----
        h1 = acts.tile([128, KT, T], F8, tag="hHI", name="h1")
        ln_chunk(xT, g1, b1, CH_A, h1, None, src16=xTb)
        ln_chunk(xT, g1, b1, CH_B, h1, None, src16=xTb)

        qkT = acts.tile([128, 16, T], BF16, tag="big32", name="qkT")

        def qk_cb(ps, ftile, ci, c0, cw):
            nc.scalar.activation(out=qkT[:, ftile, c0:c0 + cw],
                                 in_=ps[:, :cw], func=AF.Identity,
                                 scale=PS_INV,
                                 bias=bias_qk[:, ftile:ftile + 1])

        vsb = acts.tile([128, ST, H * 65], F8, tag="vsb", name="vsb")
        vwb = {}
        emit(proj_units([(dd["w_qkv"][:, :], h1)], 0, 2 * C, KT, [CH_A], qk_cb))
        vproj_load(dd["w_qkv"][:, :], 2 * C, vwb)()
        emit(vproj_units(h1, range(0, 4), vsb, vwb))
        for k in range(KT):
            nc.sync.dma_start(out=xT[:, k, :], in_=xT_src[:, k, :])

        nc.sync.dma_start(out=encT,
                          in_=dd["encT"][:, :].rearrange("(kt p) t -> p kt t",
                                                         p=128))
        kvTc = acts.tile([128, KT, IP], F8, tag="kvT", name="kvTc")
        vcsb = acts.tile([128, SI, H * 65], F8, tag="vcsb", name="vcsb")
        vwc = {}

        def kv_cb(ps, ftile, ci, c0, cw):
            nc.scalar.activation(out=kvTc[:, ftile, c0:c0 + cw],
                                 in_=ps[:, :cw], func=AF.Identity,
                                 scale=PS_INV,
                                 bias=bias_kvk[:, ftile:ftile + 1])

        # filler work for self-attention chunk A: rest of chunk B's
        # projections and the encoder K/V
        pending.extend(
            proj_units([(dd["w_qkv"][:, :], h1)], 0, 2 * C, KT, [CH_B], qk_cb))
        pending.extend(vproj_units(h1, range(4, 8), vsb, vwb))

        attnT = acts.tile([128, KT, T], F8, tag="attnT", name="attnT")
        attention(qkT, qkT[:, 8:16, :], vsb, CH_A, True, attnT, per_pair=4)
        drain()
        emit(proj_units([(dd["w_ao"][:, :], attnT)], 0, C, KT, [CH_A],
                        resid_cb(bias_ao, bao_nz, PS_INV)))

        h2 = acts.tile([128, KT, T], F8, tag="hHI", name="h2")
        pending.extend(
            proj_units([(dd["w_kv"][:, :], encT)], 0, C, KT, ECH, kv_cb))
        pending.append(vproj_load(dd["w_kv"][:, :], C, vwc))
        pending.extend(vproj_units(encT, range(SI), vcsb, vwc,
                                   bias_t=bvcb if bkv_nz else None,
                                   pad_mask=smask, last_s=SI - 1))
        pending.append(lambda: ln_chunk(xT, g2, b2, CH_A, h2, None))
        attention(qkT, qkT[:, 8:16, :], vsb, CH_B, True, attnT, per_pair=3)
        drain()
        emit(proj_units([(dd["w_ao"][:, :], attnT)], 0, C, KT, [CH_B],
                        resid_cb(bias_ao, bao_nz, PS_INV)))

        # ---- sublayer 2: cross-attention ----
        ln_chunk(xT, g2, b2, CH_B, h2, None)
        q2T = acts.tile([128, KT, T], BF16, tag="big32", name="q2T")

        def q2_cb(ps, ftile, ci, c0, cw):
            nc.scalar.activation(out=q2T[:, ftile, c0:c0 + cw], in_=ps[:, :cw],
                                 func=AF.Identity, scale=PS_INV,
                                 bias=bias_q[:, ftile:ftile + 1])
        emit(proj_units([(dd["w_q"][:, :], h2)], 0, C, KT, [CH_A], q2_cb))
        pending.extend(
            proj_units([(dd["w_q"][:, :], h2)], 0, C, KT, [CH_B], q2_cb))

        attnTc = acts.tile([128, KT, T], F8, tag="attnT", name="attnTc")
        attention(q2T, kvTc, vcsb, CH_A, False, attnTc, per_pair=3)
        drain()
        emit(proj_units([(dd["w_co"][:, :], attnTc)], 0, C, KT, [CH_A],
                        resid_cb(bias_co, bco_nz, PS_INV)))

        h3 = acts.tile([128, KT, T], F8, tag="hHI", name="h3")
        h3lo = acts.tile([128, KT, T], F8E5, tag="hLO", name="h3lo")
        geluC = acts.tile([128, KT_FC, T], F8, tag="big32", name="geluC")
        geluL = acts.tile([128, KT_FC, T], F8E5, tag="attnT", name="geluL")

        def fc_cb(ps, ftile, ci, c0, cw):
            gbf = tmps.tile([128, CHW], BF16, tag="gbf", bufs=3, name="gbf")
            nc.scalar.activation(out=gbf[:, :cw], in_=ps[:, :cw],
                                 func=AF.Gelu_apprx_tanh, scale=PS_INV,
                                 bias=bias_fc[:, ftile:ftile + 1])
            nc.gpsimd.tensor_copy(out=geluC[:, ftile, c0:c0 + cw],
                                  in_=gbf[:, :cw])
            nc.vector.tensor_sub(out=geluL[:, ftile, c0:c0 + cw],
                                 in0=gbf[:, :cw],
                                 in1=geluC[:, ftile, c0:c0 + cw])
        fc_units = proj_units([(dd["w_fc"][:, :], h3), (dd["w_fc_lo"][:, :], h3),
                               (dd["w_fc"][:, :], h3lo)],
                              0, FC, KT, TCH, fc_cb, wp=wpfc, wtag="wfc")
        fc_pre = [fc_units.pop(9), fc_units.pop(0)]   # fb1 load, fb0 load
        pending.append(lambda: ln_chunk(xT, g3, b3, CH_A, h3, h3lo))
        pending.extend(reversed(fc_pre))
        attention(q2T, kvTc, vcsb, CH_B, False, attnTc, per_pair=3)
        drain()
        emit(proj_units([(dd["w_co"][:, :], attnTc)], 0, C, KT, [CH_B],
                        resid_cb(bias_co, bco_nz, PS_INV)))

        # ---- sublayer 3: MLP (fc/mo fully compensated fp8, full-T) ----
        ln_chunk(xT, g3, b3, CH_B, h3, h3lo)
        emit(fc_units)
        # gelu output is at natural scale -> mo psum scale is SW only
        emit(proj_units([(dd["w_mo_pt"][:, :], geluC),
                         (dd["w_mo_lo_pt"][:, :], geluC)],
                        0, C, KT_FC, TCH,
                        resid_cb(bias_mo, bmo_nz, 1.0 / SW, store_out=True),
                        fbw=128, wp=wpmo, wtag="wmo", pretiled=True))


def _build(flags):
    nc = bass.Bass()
    dd = {}

    def inp(name, shape, dt):
        dd[name] = nc.dram_tensor(name, shape, dt, kind="ExternalInput")
        return dd[name]

    inp("xT", [C, T], F32)
    inp("xTb", [C, T], BF16)
    inp("encT", [C, IP], F8)
    inp("w_qkv", [C, 3 * C], F8)
    inp("w_ao", [C, C], F8)
    inp("w_q", [C, C], F8)
    inp("w_kv", [C, 2 * C], F8)
    inp("w_co", [C, C], F8)
    inp("w_fc", [C, FC], F8)
    inp("w_fc_lo", [C, FC], F8)
    inp("w_mo_pt", [KT * 128, KT_FC * 128], F8)
    inp("w_mo_lo_pt", [KT * 128, KT_FC * 128], F8)
    for n, sz in [("b_qkv", 3 * C), ("b_q", C), ("b_kv", 2 * C), ("b_ao", C),
                  ("b_co", C), ("b_fc", FC), ("b_mo", C),
                  ("ln1_g", C), ("ln1_b", C), ("ln2_g", C), ("ln2_b", C),
                  ("ln3_g", C), ("ln3_b", C), ("bvb16", C), ("bvcb16", C)]:
        inp(n, [sz], F32)
    inp("trineg", [128, 128], BF16)
    inp("identm", [128, 128], BF16)
    inp("allneg", [128, 128], BF16)
    inp("smask", [128, 1], F32)
    inp("onesc", [128, 128], BF16)
    inp("ebk", [2, 128], BF16)
    o = nc.dram_tensor("o", [C, T], F32, kind="ExternalOutput")

    with tile.TileContext(nc) as tc:
        _emit(nc, tc, dd, o, flags)
    return nc


_BUILT = None


def _get_built(flags):
    global _BUILT
    if _BUILT is None or _BUILT[0] != flags:
        _BUILT = (flags, _build(flags))
    return _BUILT[1]


def make_inmaps(inputs):
    e4 = ml_dtypes.float8_e4m3
    bf = ml_dtypes.bfloat16
    x = np.asarray(inputs["x"], np.float32)
    enc = np.asarray(inputs["encoder_output"], np.float32)
    shared = {}
    for wn in ["w_qkv", "w_ao", "w_q", "w_kv", "w_co", "w_fc", "w_mo"]:
        wf = np.ascontiguousarray(np.asarray(inputs[wn], np.float32)) * SW
        hi = wf.astype(e4)
        shared[wn] = hi
        if wn in ("w_fc", "w_mo"):
            shared[wn + "_lo"] = (wf - hi.astype(np.float32)).astype(e4)

    def pretile(a):
        # a[i, o]: i = kt*128 + p (kt over KT_FC), o = fb*128 + c (fb over 8)
        t = a.reshape(KT_FC, 128, KT, 128).transpose(2, 1, 0, 3)
        return np.ascontiguousarray(t.reshape(KT * 128, KT_FC * 128))

    shared["w_mo_pt"] = pretile(shared.pop("w_mo"))
    shared["w_mo_lo_pt"] = pretile(shared.pop("w_mo_lo"))
    for bn in ["b_qkv", "b_q", "b_kv", "b_ao", "b_co", "b_fc", "b_mo",
               "ln1_g", "ln2_g", "ln3_g"]:
        shared[bn] = np.ascontiguousarray(np.asarray(inputs[bn], np.float32))
    for bn in ["ln1_b", "ln2_b", "ln3_b"]:
        shared[bn] = np.ascontiguousarray(
            np.asarray(inputs[bn], np.float32)) * SA
    shared["bvb16"] = np.asarray(inputs["b_qkv"], np.float32)[2 * C:] * SA
    shared["bvcb16"] = np.asarray(inputs["b_kv"], np.float32)[C:] * SA
    shared["trineg"] = (np.triu(np.ones((128, 128), np.float32), 1)
                        * -2400.0).astype(bf)
    shared["identm"] = np.eye(128, dtype=np.float32).astype(bf)
    shared["allneg"] = np.full((128, 128), -2400.0, np.float32).astype(bf)
    sm = np.zeros((128, 1), np.float32)
    sm[:I - 2 * 128, 0] = 1.0
    shared["smask"] = sm
    shared["onesc"] = np.ones((128, 128), bf)
    eb = np.zeros((2, 128), np.float32)
    eb[0, 0:64] = 1.0
    eb[1, 64:128] = 1.0
    shared["ebk"] = eb.astype(bf)
    in_maps = []
    for c in range(B):
        m = dict(shared)
        m["xT"] = np.ascontiguousarray(x[c].T)
        m["xTb"] = m["xT"].astype(bf)
        eT = np.zeros((C, IP), np.float32)
        eT[:, :I] = enc[c].T * SA
        m["encT"] = eT.astype(e4)
        in_maps.append(m)
    return in_maps


def kernel(**inputs):
    ln_trivial = all(
        np.all(np.asarray(inputs[f"ln{i}_g"]) == 1.0)
        and not np.any(np.asarray(inputs[f"ln{i}_b"])) for i in (1, 2, 3))
    flags = tuple(
        bool(np.any(np.asarray(inputs[n])))
        for n in ("b_qkv", "b_q", "b_kv", "b_ao", "b_co", "b_fc", "b_mo")
    ) + (ln_trivial,)
    nc = _get_built(flags)
    in_maps = make_inmaps(inputs)
    res = run_bass_kernel_spmd(nc, in_maps, core_ids=list(range(B)))
    out = np.stack([np.ascontiguousarray(res.results[c]["o"].T)
                    for c in range(B)]).astype(np.float32)
    return out
